# revision 7
# baseline (speedup 1.0000x reference)
import sys, os
for _p in ("/opt/trn_rl_repo", os.path.expanduser("~/.axon_site/_ro/trn_rl_repo")):
    if os.path.isdir(_p) and _p not in sys.path:
        sys.path.insert(0, _p)

import numpy as np
import ml_dtypes

import concourse.bass as bass
import concourse.bacc as bacc
import concourse.mybir as mybir
from concourse.bass_utils import run_bass_kernel_spmd
from concourse.tile import TileContext

F32 = mybir.dt.float32
BF16 = mybir.dt.bfloat16
AF = mybir.ActivationFunctionType
ALU = mybir.AluOpType
bf = ml_dtypes.bfloat16

B, T, I, H, L = 64, 512, 256, 768, 2
NCORES = 8
BL = B // NCORES          # 8 batch rows per core
KT = H // 128             # 6 k-tiles over H
KI = I // 128             # 2 k-tiles over I
BT = T * BL               # 4096 (t-major, col = 8*t + b)
G4 = 4 * H                # 3072
# gate order used on-device: 0=i, 1=f, 2=o, 3=g  (sigmoid gates first)
# pytorch order in weights:  i, f, g, o
PT_GATE = [0, 1, 3, 2]
GSCALE = [0.5, 0.5, 0.5, 1.0]   # tanh-trick prescale for sigmoid gates

_CACHE = {}


def _build_nc(unroll=4):
    nc = bacc.Bacc("TRN2", target_bir_lowering=False, debug=False, num_devices=NCORES)

    d_xT = nc.dram_tensor("xT", [128, KI * BT], BF16, kind="ExternalInput")
    d_wih0 = nc.dram_tensor("wih0", [128, KI * G4], BF16, kind="ExternalInput")
    d_wih1 = nc.dram_tensor("wih1", [128, KT * G4], BF16, kind="ExternalInput")
    d_whh0 = nc.dram_tensor("whh0", [128, 4 * KT * H], BF16, kind="ExternalInput")
    d_whh1 = nc.dram_tensor("whh1", [128, 4 * KT * H], BF16, kind="ExternalInput")
    d_b0 = nc.dram_tensor("b0", [128, G4], F32, kind="ExternalInput")
    d_b1 = nc.dram_tensor("b1", [128, G4], F32, kind="ExternalInput")
    d_id = nc.dram_tensor("ident", [128, 128], F32, kind="ExternalInput")

    o_ys = nc.dram_tensor("ys", [128, KT * BT], BF16, kind="ExternalOutput")
    o_h0 = nc.dram_tensor("h0T", [128, 48], BF16, kind="ExternalOutput")
    o_c0 = nc.dram_tensor("c0T", [128, 48], F32, kind="ExternalOutput")
    o_c1 = nc.dram_tensor("c1T", [128, 48], F32, kind="ExternalOutput")

    g_gx0 = nc.dram_tensor("gxbuf0", [T, 128, H], F32)
    g_gx1 = nc.dram_tensor("gxbuf1", [T, 128, H], F32)

    with TileContext(nc) as tc:
        with tc.tile_pool(name="glob", bufs=1) as gp:
            s_id = gp.tile([128, 128], F32)
            s_ys0 = gp.tile([128, KT * BT], BF16)   # layer0 output, transposed (h-part, k*BT + 8t + b)
            s_ys1 = gp.tile([128, KT * BT], BF16)
            s_hT = gp.tile([128, 192], BF16)        # h state, transposed, 4 replicas (k*32 + r*8 + b)
            s_c = gp.tile([128, 48], F32)           # c state, transposed
            nc.sync.dma_start(out=s_id[:], in_=d_id[:])

            # ---------- input projection: gx = x @ W_ih.T + bias -> DRAM ----------
            def inproj(d_w, d_b, lhs_tile_fn, nk, g_gx):
                with tc.tile_pool(name="ip", bufs=1) as wp, \
                     tc.tile_pool(name="ipl", bufs=3) as lp, \
                     tc.tile_pool(name="ipp", bufs=2, space="PSUM") as pp:
                    s_w = wp.tile([128, nk * G4], BF16)
                    s_b = wp.tile([128, G4], F32)
                    nc.sync.dma_start(out=s_w[:], in_=d_w[:])
                    nc.sync.dma_start(out=s_b[:], in_=d_b[:])
                    for m in range(BT // 128):      # 32 m-tiles of 128 (t,b) rows
                        s_gx = lp.tile([128, G4], F32, tag="gx")
                        for n in range(G4 // 512):  # 6 n-chunks
                            p = pp.tile([128, 512], F32, tag="pg")
                            for k in range(nk):
                                nc.tensor.matmul(
                                    p[:],
                                    lhsT=lhs_tile_fn(k, m),
                                    rhs=s_w[:, k * G4 + 512 * n: k * G4 + 512 * (n + 1)],
                                    start=(k == 0), stop=(k == nk - 1),
                                )
                            nc.vector.tensor_add(
                                s_gx[:, 512 * n:512 * (n + 1)], p[:],
                                s_b[:, 512 * n:512 * (n + 1)])
                        # scatter to DRAM: rows (t,b) + free (g,h) -> [t, 32g+b, h]
                        for tp in range(16):
                            src = s_gx[8 * tp:8 * tp + 8, :]
                            dst = g_gx[16 * m + tp] \
                                .rearrange("(g r) h -> g r h", g=4, r=32)[:, 0:8, :] \
                                .rearrange("g b h -> b g h")
                            nc.sync.dma_start(out=dst, in_=src)

            # ---------- one recurrence step ----------
            def step(t, s_w, g_gx, s_ysT, lpool, psp):
                s_gx = lpool.tile([128, H], F32, tag="gx")
                nc.sync.dma_start(out=s_gx[:], in_=g_gx[bass.ds(t, 1)].squeeze(0))
                p_g = psp.tile([128, H], F32, tag="pg")
                hT4 = s_hT[:].rearrange("p (k rb) -> p k rb", k=KT, rb=32)
                for k in range(KT):
                    for (n0, n1) in ((0, 512), (512, H)):
                        for g in range(4):
                            nc.tensor.matmul(
                                p_g[32 * g:32 * (g + 1), n0:n1],
                                lhsT=hT4[:, k],
                                rhs=s_w[:, (g * KT + k) * H + n0:(g * KT + k) * H + n1],
                                start=(k == 0), stop=(k == KT - 1),
                                tile_position=(0, 32 * g),
                                skip_group_check=True,
                            )
                s_gpre = lpool.tile([128, H], F32, tag="gpre")
                nc.vector.tensor_add(s_gpre[:], p_g[:], s_gx[:])
                p_T = psp.tile([128, H], F32, tag="pT")
                for k in range(KT):
                    nc.tensor.matmul(p_T[:, 128 * k:128 * (k + 1)],
                                     lhsT=s_gpre[:, 128 * k:128 * (k + 1)], rhs=s_id[:],
                                     is_transpose=True, skip_group_check=True)
                s_tall = lpool.tile([128, 192], F32, tag="tall")
                pT_g = p_T[:].rearrange("p (k g b) -> p k g b", k=KT, g=4, b=32)[:, :, :, 0:8]
                tall4 = s_tall[:].rearrange("p (k g b) -> p k g b", k=KT, g=4, b=8)
                nc.scalar.activation(tall4, pT_g, AF.Tanh)
                ifo = s_tall[:].rearrange("p (k g b) -> p k g b", k=KT, g=4, b=8)[:, :, 0:3, :]
                nc.vector.tensor_scalar(ifo, ifo, 0.5, 0.5, ALU.mult, ALU.add)
                t4 = s_tall[:].rearrange("p (k g b) -> p k g b", k=KT, g=4, b=8)
                s_tmp = lpool.tile([128, 48], F32, tag="tmp")
                tmp3 = s_tmp[:].rearrange("p (k b) -> p k b", k=KT, b=8)
                nc.vector.tensor_mul(tmp3, t4[:, :, 0, :], t4[:, :, 3, :])   # i*g
                s_c2 = lpool.tile([128, 48], F32, tag="c2")
                c23 = s_c2[:].rearrange("p (k b) -> p k b", k=KT, b=8)
                nc.vector.tensor_mul(c23, s_c[:].rearrange("p (k b) -> p k b", k=KT, b=8),
                                     t4[:, :, 1, :])                          # f*c
                nc.vector.tensor_add(s_c[:], s_c2[:], s_tmp[:])               # c
                s_tc = lpool.tile([128, 48], F32, tag="tc")
                nc.scalar.activation(s_tc[:], s_c[:], AF.Tanh)
                hT4o = s_hT[:].rearrange("p (k r b) -> p k r b", k=KT, r=4, b=8)
                o_sl = s_tall[:].rearrange("p (k g b) -> p k g b", k=KT, g=4, b=8)[:, :, 2, :]
                o_b = o_sl.unsqueeze(2).broadcast_to((128, KT, 4, 8))
                tc_b = s_tc[:].rearrange("p (k b) -> p k b", k=KT, b=8) \
                    .unsqueeze(2).broadcast_to((128, KT, 4, 8))
                nc.vector.tensor_mul(hT4o, o_b, tc_b)
                # record h_t into ysT history (dynamic col offset 8t)
                ys4 = s_ysT[:].rearrange("p (k c) -> p k c", k=KT, c=BT)
                nc.vector.tensor_copy(ys4[:, :, bass.ts(t, 8)],
                                      s_hT[:].rearrange("p (k r b) -> p k r b", k=KT, r=4, b=8)[:, :, 0, :])

            def recurrence(d_whh, g_gx, s_ysT):
                with tc.tile_pool(name="rw", bufs=1) as wp, \
                     tc.tile_pool(name="rl", bufs=2) as lpool, \
                     tc.tile_pool(name="rp", bufs=2, space="PSUM") as psp:
                    s_w = wp.tile([128, 4 * KT * H], BF16)
                    nc.sync.dma_start(out=s_w[:], in_=d_whh[:])
                    nc.vector.memset(s_hT[:], 0.0)
                    nc.vector.memset(s_c[:], 0.0)
                    if unroll > 1:
                        tc.For_i_unrolled(0, T, 1,
                                          lambda iv: step(iv, s_w, g_gx, s_ysT, lpool, psp),
                                          max_unroll=unroll)
                    else:
                        with tc.For_i(0, T, 1) as t:
                            step(t, s_w, g_gx, s_ysT, lpool, psp)

            # ---------- layer 0 ----------
            with tc.tile_pool(name="x0", bufs=1) as xp:
                s_xT = xp.tile([128, KI * BT], BF16)
                nc.sync.dma_start(out=s_xT[:], in_=d_xT[:])
                inproj(d_wih0, d_b0,
                       lambda k, m: s_xT[:, k * BT + 128 * m: k * BT + 128 * (m + 1)],
                       KI, g_gx0)
            recurrence(d_whh0, g_gx0, s_ys0)
            nc.sync.dma_start(out=o_h0[:], in_=s_hT[:].rearrange("p (k r b) -> p k r b", k=KT, r=4, b=8)[:, :, 0, :])
            nc.sync.dma_start(out=o_c0[:], in_=s_c[:])

            # ---------- layer 1 ----------
            inproj(d_wih1, d_b1,
                   lambda k, m: s_ys0[:, k * BT + 128 * m: k * BT + 128 * (m + 1)],
                   KT, g_gx1)
            recurrence(d_whh1, g_gx1, s_ys1)
            nc.sync.dma_start(out=o_c1[:], in_=s_c[:])
            nc.sync.dma_start(out=o_ys[:], in_=s_ys1[:])
    nc.compile()
    return nc


def _get_nc():
    if "nc" not in _CACHE:
        _CACHE["nc"] = _build_nc()
    return _CACHE["nc"]


def _prep_weights(w_ih, w_hh, b_ih, b_hh, nk):
    """Device layouts: wih [128, nk*G4] (block k: cols (g,h), prescaled);
    whh [128, 4*KT*H] (block (g,k) = scale* W_hh[ptg] [:, 128k:+128].T);
    bias [128, G4] broadcast rows."""
    wih = np.zeros((128, nk * G4), dtype=bf)
    whh = np.zeros((128, 4 * KT * H), dtype=bf)
    bvec = np.zeros(G4, dtype=np.float32)
    for g in range(4):
        pg = PT_GATE[g]
        s = GSCALE[g]
        wg_ih = w_ih[pg * H:(pg + 1) * H]       # [H, in]
        wg_hh = w_hh[pg * H:(pg + 1) * H]       # [H, H]
        for k in range(nk):
            wih[:, k * G4 + g * H:(k * G4 + (g + 1) * H)] = \
                (s * wg_ih[:, 128 * k:128 * (k + 1)].T).astype(bf)
        for k in range(KT):
            whh[:, (g * KT + k) * H:(g * KT + k + 1) * H] = \
                (s * wg_hh[:, 128 * k:128 * (k + 1)].T).astype(bf)
        bvec[g * H:(g + 1) * H] = s * (b_ih[pg * H:(pg + 1) * H] + b_hh[pg * H:(pg + 1) * H])
    bias = np.broadcast_to(bvec, (128, G4)).copy()
    return wih, whh, bias


def _unpack_T(a):
    """[128, 48] (p, k*8+b) -> [BL, H] float32"""
    r = np.zeros((BL, H), np.float32)
    a = np.asarray(a, dtype=np.float32)
    for k in range(KT):
        r[:, 128 * k:128 * (k + 1)] = a[:, 8 * k:8 * k + 8].T
    return r


def kernel(x, w_ih0, w_hh0, b_ih0, b_hh0, w_ih1, w_hh1, b_ih1, b_hh1):
    x = np.asarray(x, np.float32)
    wih0, whh0, bias0 = _prep_weights(np.asarray(w_ih0, np.float32), np.asarray(w_hh0, np.float32),
                                      np.asarray(b_ih0, np.float32), np.asarray(b_hh0, np.float32), KI)
    wih1, whh1, bias1 = _prep_weights(np.asarray(w_ih1, np.float32), np.asarray(w_hh1, np.float32),
                                      np.asarray(b_ih1, np.float32), np.asarray(b_hh1, np.float32), KT)
    ident = np.eye(128, dtype=np.float32)

    nc = _get_nc()
    in_maps = []
    for ci in range(NCORES):
        xs = x[ci * BL:(ci + 1) * BL]                 # [8, 512, 256]
        xt = xs.transpose(2, 1, 0).reshape(I, BT)     # [256, 4096] col=8t+b
        xT = np.concatenate([xt[128 * k:128 * (k + 1)] for k in range(KI)], axis=1).astype(bf)
        in_maps.append({
            "xT": xT, "wih0": wih0, "wih1": wih1, "whh0": whh0, "whh1": whh1,
            "b0": bias0, "b1": bias1, "ident": ident,
        })
    res = run_bass_kernel_spmd(nc, in_maps, core_ids=list(range(NCORES)))

    ys = np.zeros((B, T, H), np.float32)
    h0 = np.zeros((B, H), np.float32)
    h1 = np.zeros((B, H), np.float32)
    c0 = np.zeros((B, H), np.float32)
    c1 = np.zeros((B, H), np.float32)
    for ci in range(NCORES):
        r = res.results[ci]
        a = np.asarray(r["ys"], dtype=np.float32).reshape(128, KT, T, BL)
        ys[ci * BL:(ci + 1) * BL] = a.transpose(3, 2, 1, 0).reshape(BL, T, H)
        h0[ci * BL:(ci + 1) * BL] = _unpack_T(r["h0T"])
        c0[ci * BL:(ci + 1) * BL] = _unpack_T(r["c0T"])
        c1[ci * BL:(ci + 1) * BL] = _unpack_T(r["c1T"])
    h1[:] = ys[:, T - 1, :]
    return ys, ((h0, h1), (c0, c1))
